# revision 34
# baseline (speedup 1.0000x reference)
"""Distributed causal multi-head attention (RoPE) for 8 TRN2 NeuronCores.

Problem: B=4, S=2048, D=2048, H=16 heads, DH=128.
Sharding: 2D - data-parallel over the 4 batches x tensor-parallel over 2
head-groups of 8 heads (Megatron-style: Wqkv column-sharded per head
group, Wo row-sharded).  Core c handles batch c//2, head group c%2.
Each core returns a partial output projection [S, D]; the host sums the
two group partials per batch (the "all-reduce") and stacks batches.

Design (evolved from a 728us fp32r baseline via trace analysis):
  * bf16 everywhere on device (x, weights, Q/K/V, scores, z).  bf16
    matmuls issue at 216ns/N=512 on HW -- faster than fp32r (227) and
    fp16 (259, which hits a slow weight path).  Halved DMA/SBUF lets
    Q/K stay SBUF-resident between projection and attention (the
    baseline spilled 25MB to DRAM; its reload caused a 13us PE stall
    and a 17us HAM cold-clock window at the stage boundary).
  * 80 no-dep warmup matmuls at t=0 keep the PE HAM at 2.4GHz while the
    x DMA streams in (~25us at the ~410GB/s aggregate DMA roof); the
    HAM stays at 8/8 for the whole kernel.
  * Stage 2 was ACT-bound (one 570ns ACTIVATE per 512-wide score tile;
    ACT 97% busy while PE waited).  Two fixes: (1) score tiles for
    adjacent key blocks land side-by-side in a 2-bank [128,1024] PSUM
    pair and are exp'd by a single ACTIVATE; (2) four pure-subdiagonal
    pairs per head are exp'd on the otherwise-idle DVE instead, via a
    Schraudolph bit-trick -- one fused mul-add to int16 whose rounded
    result IS the bf16 bit pattern of exp (constants pre-divided by
    2^16), written through a bitcast view.  ~1.5% RMS exp error on
    ~23% of the attention mass; measured end-to-end rel err 7.3e-3.
  * DVE micro-op count cut: persistent double-parity vau tiles (the
    softmax-denominator ones-column is written once, not per head),
    reciprocal batched over qa-pairs (PV accumulates two adjacent
    query blocks into one 2-bank PSUM tile), and transposes gathered
    in quads so zT is evicted with one 512-wide copy per q-chunk.
    The transpose quads write into a bf16 bitcast view of an spp-tag
    PSUM slot, freeing a bank pair so the QK score pairs triple-buffer
    (stage 2: 184us ACT-bound -> 160us with ACT/DVE/PE all ~85%).
  * Exact causal trim on QK (bf16 has no >=256 moving-dim constraint).
  * QK emission runs 4 PV-groups ahead of consumption, across head
    boundaries (PV-first loop order makes this WAR-safe with the
    single-buffered score tags), so exp latency hides under PE work.
    Stage 2 ends up PE-bound with ~12us of residual gaps.

Per-core pipeline:
  stage 1: QKV projection from xT (d-major).  RoPE fused into the PSUM
           eviction for Q/K (DVE, rotate_half via cross-partition reads,
           sign folded into the sin table) -> bf16 SBUF tiles.  V evicted
           by ACT to a small DRAM scratch (SBUF budget), streamed back
           per head in stage 2 with double-buffered prefetch.
  stage 2: per head: scoresT[k,q] = K-tile.T x Q into PSUM pairs, exp via
           ACT or DVE-Schraudolph into bf16 tiles, triangular mask
           multiply on diagonal blocks only, PV via bf16 matmuls with a
           fused ones-column giving the softmax denominator for free,
           then reciprocal-scale + PE transpose into resident zT tiles.
  stage 3: output projection out = sum_h zT_h.T x WoT_h (N=512 chains),
           Wo prefetched during stage 2.

Measured: ~631-638us HW exec across 8 cores (was 728us), rel err ~7.3e-3
vs the fp32 reference (gate 2e-2).

NOTE on emission-order hazards (bugs hit during development): with
persistent tiles (vau, cos/sin tables), the Tile framework's
dependencies follow EMISSION order -- a consumer emitted after a
producer DMA reads the NEW data even if logically it wanted the old.
vau re-DMA for head h+2 must therefore be emitted only after ALL of
head h's PV groups; the cos/sin DMA must be emitted before the first
eviction that reads it.  GPSIMD tensor ops were tried and rejected:
CAST on GPSIMD is 4.3us per [128,1024] tile (7x DVE), and stride-0
broadcast_to on DVE tensor_tensor produced wrong results.
"""

import sys

if '/opt/trn_rl_repo' not in sys.path:
    sys.path.insert(0, '/opt/trn_rl_repo')

import math

import ml_dtypes
import numpy as np

B, S, D, H, DH = 4, 2048, 2048, 16, 128
BASE = 10000.0
P = 128
NT = S // P          # 16 token tiles of 128
NC512 = S // 512     # 4 token chunks of 512
NDM = D // P         # 16 d_model chunks
HG = 8               # heads per group
SCALE = 1.0 / math.sqrt(DH)
NWARM = 80           # warmup matmuls to hold HAM at 2.4GHz during x load

_CACHE = {}


def _build_program():
    import concourse.bacc as bacc
    import concourse.mybir as mybir
    from concourse.tile import TileContext
    from concourse.masks import make_identity

    F32 = mybir.dt.float32
    F16 = mybir.dt.bfloat16
    I16 = mybir.dt.int16
    EXP = mybir.ActivationFunctionType.Exp
    MUL = mybir.AluOpType.mult
    ADD = mybir.AluOpType.add
    # Schraudolph exp-by-bit-trick constants for the DVE-offloaded score
    # tiles: f32 bits of exp(s*SCALE) ~ int(A*s + B).  Divided by 2^16 and
    # converted to int16, the rounded top half IS the bf16 bit pattern, so
    # one DVE mul-add writes exp() straight into the bf16 tile via bitcast.
    SCH_A = 12102203.161561485 * SCALE / 65536.0
    SCH_B = 1064986316.5 / 65536.0

    nc = bacc.Bacc('TRN2', target_bir_lowering=False, debug=False, num_devices=8)

    # ---- DRAM I/O (all fp16 except tables and out) ----
    xT = nc.dram_tensor('xT', [P, NDM, S], F16, kind='ExternalInput').ap()
    wqkT = nc.dram_tensor('wqkT', [2 * HG, P, NDM, P], F16, kind='ExternalInput').ap()
    wvT = nc.dram_tensor('wvT', [P, NDM, HG * P], F16, kind='ExternalInput').ap()
    woT = nc.dram_tensor('woT', [P, HG, D], F16, kind='ExternalInput').ap()
    cosT = nc.dram_tensor('cosT', [P, S], F32, kind='ExternalInput').ap()
    sinP = nc.dram_tensor('sinP', [P, S], F32, kind='ExternalInput').ap()
    maskT = nc.dram_tensor('maskT', [P, P], F16, kind='ExternalInput').ap()
    onesb = nc.dram_tensor('onesb', [P, 1], F16, kind='ExternalInput').ap()
    out = nc.dram_tensor('out', [NT, P, D], F32, kind='ExternalOutput').ap()

    # ---- DRAM scratch: V only (Q/K stay in SBUF) ----
    v_scr = nc.dram_tensor('v_scr', [NT, P, HG * P], F16).ap()

    with TileContext(nc) as tc:
        # outer pool: tensors that live across stage boundaries
        with tc.tile_pool(name='glob', bufs=1) as gpool:
            qk_sb = [gpool.tile([P, S], F16, name=f'qk{fb}') for fb in range(2 * HG)]
            ident = gpool.tile([P, P], F16, name='ident')
            msk = gpool.tile([P, P], F16, name='msk')
            ones_sb = gpool.tile([P, 1], F16, name='ones')
            jnk = gpool.tile([P, 512], F16, name='jnk')

            nc.gpsimd.memset(jnk[:], 0.0)
            make_identity(nc, ident[:])
            nc.sync.dma_start(ones_sb[:], onesb[:])

            # persistent double-parity vau tiles; the softmax-denominator
            # ones column is written once, only the V data re-DMAs per head
            vau_sb = [[gpool.tile([P, P + 1], F16, name=f'vau_{kt}_{par}')
                       for par in range(2)] for kt in range(NT)]
            for kt in range(NT):
                for par in range(2):
                    nc.vector.tensor_copy(vau_sb[kt][par][:, P:P + 1],
                                          ones_sb[:])

            def fetch_vau(h):
                for kt in range(NT):
                    nc.sync.dma_start(vau_sb[kt][h & 1][:, 0:P],
                                      v_scr[kt][:, h * P:(h + 1) * P])

            # ================= stage 1: QKV projection =================
            with tc.tile_pool(name='s1x', bufs=1) as xpool, \
                 tc.tile_pool(name='s1w', bufs=2) as wpool, \
                 tc.tile_pool(name='s1e', bufs=2) as epool, \
                 tc.tile_pool(name='s1ev', bufs=3) as evpool, \
                 tc.tile_pool(name='s1p', bufs=4, space='PSUM') as ppool:
                # warmup matmuls: no deps, stream immediately, keep HAM warm
                # while the x DMA fills SBUF (~20us)
                jps = ppool.tile([P, 512], F32, tag='pqk', name='jps')
                for w in range(NWARM):
                    nc.tensor.matmul(jps[:], ident[:], jnk[:],
                                     start=True, stop=True)

                xsb = xpool.tile([P, NDM, S], F16)
                cos_sb = xpool.tile([P, S], F32)
                sin_sb = xpool.tile([P, S], F32)

                for fb in range(2 * HG):
                    wsb = wpool.tile([P, NDM, 512], F16, tag='w', name=f'wqk{fb}')
                    nc.sync.dma_start(wsb[:, :, 0:P], wqkT[fb])
                    if fb == 0:
                        # w0 first (small, gates the first chain), then x
                        for o in range(NDM):
                            nc.sync.dma_start(xsb[:, o, :], xT[:, o, :])
                        nc.sync.dma_start(cos_sb[:], cosT[:])
                        nc.sync.dma_start(sin_sb[:], sinP[:])
                    for tcn in range(NC512):
                        ts = slice(tcn * 512, tcn * 512 + 512)
                        ps = ppool.tile([P, 512], F32, tag='pqk', name=f'pqk_{fb}_{tcn}')
                        for o in range(NDM):
                            nc.tensor.matmul(ps[:], wsb[:, o, 0:P], xsb[:, o, ts],
                                             start=(o == 0), stop=(o == NDM - 1))
                        # RoPE fused eviction (sign folded into sinP)
                        t1 = epool.tile([P, 512], F32, tag='t1', name=f't1_{fb}_{tcn}')
                        t2 = epool.tile([P, 512], F32, tag='t2', name=f't2_{fb}_{tcn}')
                        nc.vector.tensor_mul(t1[:], ps[:], cos_sb[:, ts])
                        nc.vector.tensor_mul(t2[0:64, :], ps[64:128, :], sin_sb[0:64, ts])
                        nc.vector.tensor_mul(t2[64:128, :], ps[0:64, :], sin_sb[64:128, ts])
                        nc.vector.tensor_add(qk_sb[fb][:, ts], t1[:], t2[:])

                # --- V blocks, token-major, N=512 (4 heads per chunk) ---
                for vc in range(2):
                    vs = slice(vc * 512, vc * 512 + 512)
                    wv = wpool.tile([P, NDM, 512], F16, tag='w', name=f'wv{vc}')
                    for o in range(NDM):
                        nc.sync.dma_start(wv[:, o, :], wvT[:, o, vs])
                    for tt in range(NT):
                        psv = ppool.tile([P, 512], F32, tag='pv', name=f'pv_{vc}_{tt}')
                        for o in range(NDM):
                            nc.tensor.matmul(psv[:],
                                             xsb[:, o, tt * P:(tt + 1) * P],
                                             wv[:, o, :],
                                             start=(o == 0), stop=(o == NDM - 1))
                        vsb = evpool.tile([P, 512], F16, tag='vsb', name=f'vsb_{vc}_{tt}')
                        nc.scalar.copy(vsb[:], psv[:])
                        nc.sync.dma_start(v_scr[tt][:, vs], vsb[:])
                    if vc == 0:
                        # heads 0/1 read only the vc=0 half; prefetching here
                        # orders their DMAs after vc=0 writes only
                        fetch_vau(0)
                        fetch_vau(1)

            # ================= stage 2: attention =================
            with tc.tile_pool(name='s2zt', bufs=1) as ztpool, \
                 tc.tile_pool(name='s2st', bufs=1) as stpool, \
                 tc.tile_pool(name='s2z', bufs=4) as zpool, \
                 tc.tile_pool(name='s2wo', bufs=2) as wopool, \
                 tc.tile_pool(name='s2os', bufs=4) as ospool:
                _ps2 = [tc.tile_pool(name='s2p', bufs=3, space='PSUM'),
                        tc.tile_pool(name='s2pz', bufs=2, space='PSUM')]
                sppool, zppool = [p.__enter__() for p in _ps2]

                zT = [ztpool.tile([P, S], F16, name=f'zT{h}') for h in range(HG)]
                nc.sync.dma_start(msk[:], maskT[:])

                # st tiles: pair tag (qr, pi) holds kt=2pi (left 512) and
                # kt=2pi+1 (right 512) of q-chunk qr
                st = {}

                # pure sub-diagonal pairs whose exp runs as a Schraudolph
                # bit-trick (DVE mul-add to int32 + GPSIMD bitcast copy)
                # instead of on ACT: rebalances exp work across engines
                # (ACT was the stage-2 bottleneck, then DVE; PE now binds)
                OFFLOAD = {(2, 0), (2, 1), (3, 0), (3, 1)}

                def emit_qk(h, qr):
                    qt_h, kt_h = qk_sb[h], qk_sb[HG + h]
                    base = qr * 512
                    npair = 2 * qr + 2
                    for pi in range(npair):
                        spp = sppool.tile([P, 1024], F32, tag='spp',
                                          name=f'spp_{h}_{qr}_{pi}')
                        stt = stpool.tile([P, 1024], F16, tag=f'st{qr}_{pi}',
                                          name=f'st_{h}_{qr}_{pi}')
                        eoffs = []
                        for half in range(2):
                            kt = 2 * pi + half
                            d = kt - 4 * qr
                            qoff = 0 if d < 0 else 128 * d
                            eoffs.append(qoff)
                            nc.tensor.matmul(
                                spp[:, 512 * half + qoff:512 * half + 512],
                                kt_h[:, kt * P:(kt + 1) * P],
                                qt_h[:, base + qoff:base + 512],
                                start=True, stop=True)
                        if (qr, pi) in OFFLOAD:
                            nc.vector.tensor_scalar(stt[:].bitcast(I16), spp[:],
                                                    SCH_A, SCH_B, MUL, ADD)
                            st[(h, qr, pi)] = stt
                            continue
                        # one exp for the pair; right half junk below its own
                        # eoff is never read by PV
                        e0 = eoffs[0]
                        nc.scalar.activation(stt[:, e0:1024], spp[:, e0:1024],
                                             EXP, scale=SCALE)
                        for half in range(2):
                            kt = 2 * pi + half
                            d = kt - 4 * qr
                            if d >= 0:
                                eo = 512 * half + 128 * d
                                nc.vector.tensor_mul(stt[:, eo:eo + P],
                                                     stt[:, eo:eo + P], msk[:])
                        st[(h, qr, pi)] = stt

                pend_t = []

                def flush_t():
                    ph, pqr, zsbs = pend_t.pop(0)
                    # 4 transposes of one q-chunk gathered in a single PSUM
                    # tile (a bf16 bitcast view of an spp-tag slot, freeing a
                    # bank pair so QK pairs triple-buffer), one DVE eviction
                    ztf = sppool.tile([P, 1024], F32, tag='spp',
                                      name=f'ztp_{ph}_{pqr}')
                    ztq = ztf[:].bitcast(F16)
                    for j in range(4):
                        nc.tensor.transpose(ztq[:, j * P:(j + 1) * P],
                                            zsbs[j // 2][:, j % 2, :], ident[:])
                    # ACT eviction: the quad shares the spp rotation, so its
                    # copy gates a later QK matmul's PSUM slot; the 83%-busy
                    # DVE freed it late (the ~620ns/head stall at fixed pc)
                    nc.scalar.copy(zT[ph][:, pqr * 512:(pqr + 1) * 512],
                                   ztq[:, 0:512])

                def emit_pv(h, qr):
                    zsbs = []
                    for qp in range(2):
                        zq = zppool.tile([P, 2, P + 1], F32, tag='zps',
                                         name=f'zps_{h}_{qr}_{qp}')
                        for qs2 in range(2):
                            qa = 4 * qr + 2 * qp + qs2
                            for kt in range(qa + 1):
                                stt = st[(h, qr, kt // 2)]
                                co = 512 * (kt & 1) + (qa - 4 * qr) * P
                                nc.tensor.matmul(zq[:, qs2, :],
                                                 stt[:, co:co + P],
                                                 vau_sb[kt][h & 1][:],
                                                 start=(kt == 0), stop=(kt == qa))
                        rcp2 = zpool.tile([P, 2], F32, tag='rcp',
                                          name=f'rcp_{h}_{qr}_{qp}')
                        nc.vector.reciprocal(rcp2[:], zq[:, :, P])
                        zsb2 = zpool.tile([P, 2, P], F16, tag='zsb',
                                          name=f'zsb_{h}_{qr}_{qp}')
                        nc.vector.tensor_scalar_mul(zsb2[:, 0, :], zq[:, 0, 0:P],
                                                    rcp2[:, 0:1])
                        nc.vector.tensor_scalar_mul(zsb2[:, 1, :], zq[:, 1, 0:P],
                                                    rcp2[:, 1:2])
                        zsbs.append(zsb2)
                    pend_t.append((h, qr, zsbs))
                    if len(pend_t) > 1:
                        flush_t()

                # flat group lists; QK emitted LOOKAHEAD groups ahead of PV
                groups = [(h, qr) for h in range(HG) for qr in range(NC512)]
                LOOKAHEAD = 4
                for j in range(LOOKAHEAD):
                    emit_qk(*groups[j])
                for i, (h, qr) in enumerate(groups):
                    # PV first: keeps ready PV chains ahead of exp-throttled
                    # QK matmuls in the in-order PE queue
                    emit_pv(h, qr)
                    if i + LOOKAHEAD < len(groups):
                        emit_qk(*groups[i + LOOKAHEAD])
                    if qr == 3 and h + 2 < HG:
                        fetch_vau(h + 2)

                while pend_t:
                    flush_t()

                for p in reversed(_ps2):
                    p.__exit__(None, None, None)

                # ================= stage 3: output projection =================
                with tc.tile_pool(name='s3p', bufs=4, space='PSUM') as oppool:
                    for ec in range(NC512):
                        es = slice(ec * 512, ec * 512 + 512)
                        wo = wopool.tile([P, HG, 512], F16, tag='wo', name=f'wo{ec}')
                        for h in range(HG):
                            nc.sync.dma_start(wo[:, h, :], woT[:, h, es])
                        for tt in range(NT):
                            pso = oppool.tile([P, 512], F32, tag='pso',
                                              name=f'pso_{tt}_{ec}')
                            for h in range(HG):
                                nc.tensor.matmul(pso[:], zT[h][:, tt * P:(tt + 1) * P],
                                                 wo[:, h, :],
                                                 start=(h == 0), stop=(h == HG - 1))
                            osb = ospool.tile([P, 512], F32, tag='osb',
                                              name=f'osb_{tt}_{ec}')
                            if tt % 2 == 0:
                                nc.scalar.copy(osb[:], pso[:])
                            else:
                                nc.vector.tensor_copy(osb[:], pso[:])
                            nc.sync.dma_start(out[tt][:, es], osb[:])

    nc.compile()
    return nc


def _host_inputs(x, Wqkv, Wo):
    """Build the 8 per-core input maps."""
    F16 = ml_dtypes.bfloat16
    # RoPE tables (match reference: float32 math)
    inv_freq = (1.0 / (BASE ** (np.arange(0, DH, 2, dtype=np.float32) / DH))).astype(np.float32)
    t = np.arange(S, dtype=np.float32)
    freqs = np.einsum('i,j->ij', t, inv_freq).astype(np.float32)   # [S, 64]
    emb = np.concatenate([freqs, freqs], axis=-1)                   # [S, 128]
    cos = np.cos(emb).astype(np.float32)
    sin = np.sin(emb).astype(np.float32)
    cosT = np.ascontiguousarray(cos.T)                              # [128, S]
    sinT = np.ascontiguousarray(sin.T)
    sinP = sinT.copy()
    sinP[0:64] = -sinP[0:64]

    # triangular causal mask [128, 128] f16: keep iff k_rel <= q_rel
    maskT = (np.arange(P)[:, None] <= np.arange(P)[None, :]).astype(F16)
    onesb = np.ones((P, 1), dtype=F16)

    in_maps = []
    for c in range(8):
        b, g = c // 2, c % 2
        heads = range(HG * g, HG * g + HG)
        x_b = x[b]                                       # [S, D]
        xT = np.ascontiguousarray(
            x_b.T.reshape(NDM, P, S).transpose(1, 0, 2)).astype(F16)  # [128, 16, S]
        # Q then K feature blocks, one per head in group
        blocks = [Wqkv[h * DH:(h + 1) * DH] for h in heads] + \
                 [Wqkv[D + h * DH:D + (h + 1) * DH] for h in heads]
        wqkT = np.stack([
            np.ascontiguousarray(
                blk.T.reshape(NDM, P, P).transpose(1, 0, 2))    # [128, 16, 128]
            for blk in blocks
        ]).astype(F16)                                           # [16, 128, 16, 128]
        Wv = np.concatenate([Wqkv[2 * D + h * DH:2 * D + (h + 1) * DH] for h in heads])
        wvT = np.ascontiguousarray(
            Wv.T.reshape(NDM, P, HG * P).transpose(1, 0, 2)).astype(F16)  # [128, 16, 1024]
        Wog = Wo[:, g * HG * DH:(g + 1) * HG * DH]               # [D, 1024]
        woT = np.ascontiguousarray(
            Wog.T.reshape(HG, P, D).transpose(1, 0, 2)).astype(F16)       # [128, 8, D]
        in_maps.append({
            'xT': xT, 'wqkT': wqkT, 'wvT': wvT, 'woT': woT,
            'cosT': cosT, 'sinP': sinP, 'maskT': maskT, 'onesb': onesb,
        })
    return in_maps


def kernel(x, Wqkv, Wo):
    from concourse.bass_utils import run_bass_kernel_spmd

    if 'nc' not in _CACHE:
        _CACHE['nc'] = _build_program()
    nc = _CACHE['nc']

    in_maps = _host_inputs(np.asarray(x, dtype=np.float32),
                           np.asarray(Wqkv, dtype=np.float32),
                           np.asarray(Wo, dtype=np.float32))
    res = run_bass_kernel_spmd(nc, in_maps, core_ids=list(range(8)))
    outs = [res.results[c]['out'].reshape(S, D) for c in range(8)]
    full = np.empty((B, S, D), dtype=np.float32)
    for b in range(B):
        full[b] = outs[2 * b] + outs[2 * b + 1]
    return full


# revision 35
# speedup vs baseline: 1.0194x; 1.0194x over previous
"""Distributed causal multi-head attention (RoPE) for 8 TRN2 NeuronCores.

Problem: B=4, S=2048, D=2048, H=16 heads, DH=128.
Sharding: 2D - data-parallel over the 4 batches x tensor-parallel over 2
head-groups of 8 heads (Megatron-style: Wqkv column-sharded per head
group, Wo row-sharded).  Core c handles batch c//2, head group c%2.
Each core returns a partial output projection [S, D]; the host sums the
two group partials per batch (the "all-reduce") and stacks batches.

Design (evolved from a 728us fp32r baseline via trace analysis):
  * bf16 everywhere on device (x, weights, Q/K/V, scores, z).  bf16
    matmuls issue at 216ns/N=512 on HW -- faster than fp32r (227) and
    fp16 (259, which hits a slow weight path).  Halved DMA/SBUF lets
    Q/K stay SBUF-resident between projection and attention (the
    baseline spilled 25MB to DRAM; its reload caused a 13us PE stall
    and a 17us HAM cold-clock window at the stage boundary).
  * 80 no-dep warmup matmuls at t=0 keep the PE HAM at 2.4GHz while the
    x DMA streams in (~25us at the ~410GB/s aggregate DMA roof); the
    HAM stays at 8/8 for the whole kernel.
  * Stage 2 was ACT-bound (one 570ns ACTIVATE per 512-wide score tile;
    ACT 97% busy while PE waited).  Two fixes: (1) score tiles for
    adjacent key blocks land side-by-side in a 2-bank [128,1024] PSUM
    pair and are exp'd by a single ACTIVATE; (2) four pure-subdiagonal
    pairs per head are exp'd on the otherwise-idle DVE instead, via a
    Schraudolph bit-trick -- one fused mul-add to int16 whose rounded
    result IS the bf16 bit pattern of exp (constants pre-divided by
    2^16), written through a bitcast view.  ~1.5% RMS exp error on
    ~23% of the attention mass; measured end-to-end rel err 7.3e-3.
  * DVE micro-op count cut: persistent double-parity vau tiles (the
    softmax-denominator ones-column is written once, not per head),
    reciprocal batched over qa-pairs (PV accumulates two adjacent
    query blocks into one 2-bank PSUM tile), and transposes gathered
    in quads so zT is evicted with one 512-wide copy per q-chunk.
    The transpose quads write into a bf16 bitcast view of an spp-tag
    PSUM slot, freeing a bank pair so the QK score pairs triple-buffer
    (stage 2: 184us ACT-bound -> 160us with ACT/DVE/PE all ~85%).
  * Exact causal trim on QK (bf16 has no >=256 moving-dim constraint).
  * QK emission runs 4 PV-groups ahead of consumption, across head
    boundaries (PV-first loop order makes this WAR-safe with the
    single-buffered score tags), so exp latency hides under PE work.
    Stage 2 ends up PE-bound with ~12us of residual gaps.

Per-core pipeline:
  stage 1: QKV projection from xT (d-major).  RoPE fused into the PSUM
           eviction for Q/K (DVE, rotate_half via cross-partition reads,
           sign folded into the sin table) -> bf16 SBUF tiles.  V evicted
           by ACT to a small DRAM scratch (SBUF budget), streamed back
           per head in stage 2 with double-buffered prefetch.
  stage 2: per head: scoresT[k,q] = K-tile.T x Q into PSUM pairs, exp via
           ACT or DVE-Schraudolph into bf16 tiles, triangular mask
           multiply on diagonal blocks only, PV via bf16 matmuls with a
           fused ones-column giving the softmax denominator for free,
           then reciprocal-scale + PE transpose into resident zT tiles.
  stage 3: output projection out = sum_h zT_h.T x WoT_h (N=512 chains),
           Wo prefetched during stage 2.

Measured: ~631-638us HW exec across 8 cores (was 728us), rel err ~7.3e-3
vs the fp32 reference (gate 2e-2).

NOTE on emission-order hazards (bugs hit during development): with
persistent tiles (vau, cos/sin tables), the Tile framework's
dependencies follow EMISSION order -- a consumer emitted after a
producer DMA reads the NEW data even if logically it wanted the old.
vau re-DMA for head h+2 must therefore be emitted only after ALL of
head h's PV groups; the cos/sin DMA must be emitted before the first
eviction that reads it.  GPSIMD tensor ops were tried and rejected:
CAST on GPSIMD is 4.3us per [128,1024] tile (7x DVE), and stride-0
broadcast_to on DVE tensor_tensor produced wrong results.
"""

import sys

if '/opt/trn_rl_repo' not in sys.path:
    sys.path.insert(0, '/opt/trn_rl_repo')

import math

import ml_dtypes
import numpy as np

B, S, D, H, DH = 4, 2048, 2048, 16, 128
BASE = 10000.0
P = 128
NT = S // P          # 16 token tiles of 128
NC512 = S // 512     # 4 token chunks of 512
NDM = D // P         # 16 d_model chunks
HG = 8               # heads per group
SCALE = 1.0 / math.sqrt(DH)
NWARM = 80           # warmup matmuls to hold HAM at 2.4GHz during x load

_CACHE = {}


def _build_program():
    import concourse.bacc as bacc
    import concourse.mybir as mybir
    from concourse.tile import TileContext
    from concourse.masks import make_identity

    F32 = mybir.dt.float32
    F16 = mybir.dt.bfloat16
    I16 = mybir.dt.int16
    EXP = mybir.ActivationFunctionType.Exp
    MUL = mybir.AluOpType.mult
    ADD = mybir.AluOpType.add
    # Schraudolph exp-by-bit-trick constants for the DVE-offloaded score
    # tiles: f32 bits of exp(s*SCALE) ~ int(A*s + B).  Divided by 2^16 and
    # converted to int16, the rounded top half IS the bf16 bit pattern, so
    # one DVE mul-add writes exp() straight into the bf16 tile via bitcast.
    SCH_A = 12102203.161561485 * SCALE / 65536.0
    SCH_B = 1064986316.5 / 65536.0

    nc = bacc.Bacc('TRN2', target_bir_lowering=False, debug=False, num_devices=8)

    # ---- DRAM I/O (all fp16 except tables and out) ----
    xT = nc.dram_tensor('xT', [P, NDM, S], F16, kind='ExternalInput').ap()
    wqkT = nc.dram_tensor('wqkT', [2 * HG, P, NDM, P], F16, kind='ExternalInput').ap()
    wvT = nc.dram_tensor('wvT', [P, NDM, HG * P], F16, kind='ExternalInput').ap()
    woT = nc.dram_tensor('woT', [P, HG, D], F16, kind='ExternalInput').ap()
    cosT = nc.dram_tensor('cosT', [P, S], F32, kind='ExternalInput').ap()
    sinP = nc.dram_tensor('sinP', [P, S], F32, kind='ExternalInput').ap()
    maskT = nc.dram_tensor('maskT', [P, P], F16, kind='ExternalInput').ap()
    onesb = nc.dram_tensor('onesb', [P, 1], F16, kind='ExternalInput').ap()
    out = nc.dram_tensor('out', [NT, P, D], F32, kind='ExternalOutput').ap()

    # ---- DRAM scratch: V only (Q/K stay in SBUF) ----
    v_scr = nc.dram_tensor('v_scr', [NT, P, HG * P], F16).ap()

    with TileContext(nc) as tc:
        # outer pool: tensors that live across stage boundaries
        with tc.tile_pool(name='glob', bufs=1) as gpool:
            qk_sb = [gpool.tile([P, S], F16, name=f'qk{fb}') for fb in range(2 * HG)]
            ident = gpool.tile([P, P], F16, name='ident')
            msk = gpool.tile([P, P], F16, name='msk')
            ones_sb = gpool.tile([P, 1], F16, name='ones')
            jnk = gpool.tile([P, 512], F16, name='jnk')

            nc.gpsimd.memset(jnk[:], 0.0)
            make_identity(nc, ident[:])
            nc.sync.dma_start(ones_sb[:], onesb[:])

            # persistent double-parity vau tiles; the softmax-denominator
            # ones column is written once, only the V data re-DMAs per head
            vau_sb = [[gpool.tile([P, P + 1], F16, name=f'vau_{kt}_{par}')
                       for par in range(2)] for kt in range(NT)]
            for kt in range(NT):
                for par in range(2):
                    nc.vector.tensor_copy(vau_sb[kt][par][:, P:P + 1],
                                          ones_sb[:])

            def fetch_vau(h):
                for kt in range(NT):
                    nc.sync.dma_start(vau_sb[kt][h & 1][:, 0:P],
                                      v_scr[kt][:, h * P:(h + 1) * P])

            # ================= stage 1: QKV projection =================
            with tc.tile_pool(name='s1x', bufs=1) as xpool, \
                 tc.tile_pool(name='s1w', bufs=2) as wpool, \
                 tc.tile_pool(name='s1e', bufs=2) as epool, \
                 tc.tile_pool(name='s1ev', bufs=3) as evpool, \
                 tc.tile_pool(name='s1p', bufs=4, space='PSUM') as ppool:
                # warmup matmuls: no deps, stream immediately, keep HAM warm
                # while the x DMA fills SBUF (~20us)
                jps = ppool.tile([P, 512], F32, tag='pqk', name='jps')
                for w in range(NWARM):
                    nc.tensor.matmul(jps[:], ident[:], jnk[:],
                                     start=True, stop=True)

                xsb = xpool.tile([P, NDM, S], F16)
                cos_sb = xpool.tile([P, S], F32)
                sin_sb = xpool.tile([P, S], F32)

                for fb in range(2 * HG):
                    wsb = wpool.tile([P, NDM, 512], F16, tag='w', name=f'wqk{fb}')
                    nc.sync.dma_start(wsb[:, :, 0:P], wqkT[fb])
                    if fb == 0:
                        # w0 first (small, gates the first chain), then x
                        for o in range(NDM):
                            nc.sync.dma_start(xsb[:, o, :], xT[:, o, :])
                        nc.sync.dma_start(cos_sb[:], cosT[:])
                        nc.sync.dma_start(sin_sb[:], sinP[:])
                    for tcn in range(NC512):
                        ts = slice(tcn * 512, tcn * 512 + 512)
                        ps = ppool.tile([P, 512], F32, tag='pqk', name=f'pqk_{fb}_{tcn}')
                        for o in range(NDM):
                            nc.tensor.matmul(ps[:], wsb[:, o, 0:P], xsb[:, o, ts],
                                             start=(o == 0), stop=(o == NDM - 1))
                        # RoPE fused eviction (sign folded into sinP)
                        t1 = epool.tile([P, 512], F32, tag='t1', name=f't1_{fb}_{tcn}')
                        t2 = epool.tile([P, 512], F32, tag='t2', name=f't2_{fb}_{tcn}')
                        nc.vector.tensor_mul(t1[:], ps[:], cos_sb[:, ts])
                        nc.vector.tensor_mul(t2[0:64, :], ps[64:128, :], sin_sb[0:64, ts])
                        nc.vector.tensor_mul(t2[64:128, :], ps[0:64, :], sin_sb[64:128, ts])
                        nc.vector.tensor_add(qk_sb[fb][:, ts], t1[:], t2[:])

                # --- V blocks, token-major, N=512 (4 heads per chunk) ---
                for vc in range(2):
                    vs = slice(vc * 512, vc * 512 + 512)
                    wv = wpool.tile([P, NDM, 512], F16, tag='w', name=f'wv{vc}')
                    for o in range(NDM):
                        nc.sync.dma_start(wv[:, o, :], wvT[:, o, vs])
                    for tt in range(NT):
                        psv = ppool.tile([P, 512], F32, tag='pv', name=f'pv_{vc}_{tt}')
                        for o in range(NDM):
                            nc.tensor.matmul(psv[:],
                                             xsb[:, o, tt * P:(tt + 1) * P],
                                             wv[:, o, :],
                                             start=(o == 0), stop=(o == NDM - 1))
                        vsb = evpool.tile([P, 512], F16, tag='vsb', name=f'vsb_{vc}_{tt}')
                        nc.scalar.copy(vsb[:], psv[:])
                        nc.sync.dma_start(v_scr[tt][:, vs], vsb[:])
                    if vc == 0:
                        # heads 0/1 read only the vc=0 half; prefetching here
                        # orders their DMAs after vc=0 writes only
                        fetch_vau(0)
                        fetch_vau(1)

            # ================= stage 2: attention =================
            with tc.tile_pool(name='s2zt', bufs=1) as ztpool, \
                 tc.tile_pool(name='s2st', bufs=1) as stpool, \
                 tc.tile_pool(name='s2z', bufs=4) as zpool, \
                 tc.tile_pool(name='s2wo', bufs=2) as wopool, \
                 tc.tile_pool(name='s2os', bufs=4) as ospool:
                _ps2 = [tc.tile_pool(name='s2p', bufs=3, space='PSUM'),
                        tc.tile_pool(name='s2pz', bufs=2, space='PSUM')]
                sppool, zppool = [p.__enter__() for p in _ps2]

                zT = [ztpool.tile([P, S], F16, name=f'zT{h}') for h in range(HG)]
                nc.sync.dma_start(msk[:], maskT[:])

                # st tiles: pair tag (qr, pi) holds kt=2pi (left 512) and
                # kt=2pi+1 (right 512) of q-chunk qr
                st = {}

                # pure sub-diagonal pairs whose exp runs as a Schraudolph
                # bit-trick (DVE mul-add to int32 + GPSIMD bitcast copy)
                # instead of on ACT: rebalances exp work across engines
                # (ACT was the stage-2 bottleneck, then DVE; PE now binds)
                OFFLOAD = {(2, 0), (2, 1), (3, 0), (3, 1)}

                def emit_qk(h, qr):
                    qt_h, kt_h = qk_sb[h], qk_sb[HG + h]
                    base = qr * 512
                    npair = 2 * qr + 2
                    for pi in range(npair):
                        spp = sppool.tile([P, 1024], F32, tag='spp',
                                          name=f'spp_{h}_{qr}_{pi}')
                        stt = stpool.tile([P, 1024], F16, tag=f'st{qr}_{pi}',
                                          name=f'st_{h}_{qr}_{pi}')
                        eoffs = []
                        for half in range(2):
                            kt = 2 * pi + half
                            d = kt - 4 * qr
                            qoff = 0 if d < 0 else 128 * d
                            eoffs.append(qoff)
                            nc.tensor.matmul(
                                spp[:, 512 * half + qoff:512 * half + 512],
                                kt_h[:, kt * P:(kt + 1) * P],
                                qt_h[:, base + qoff:base + 512],
                                start=True, stop=True)
                        if (qr, pi) in OFFLOAD:
                            nc.vector.tensor_scalar(stt[:].bitcast(I16), spp[:],
                                                    SCH_A, SCH_B, MUL, ADD)
                            st[(h, qr, pi)] = stt
                            continue
                        # one exp for the pair; right half junk below its own
                        # eoff is never read by PV
                        e0 = eoffs[0]
                        nc.scalar.activation(stt[:, e0:1024], spp[:, e0:1024],
                                             EXP, scale=SCALE)
                        for half in range(2):
                            kt = 2 * pi + half
                            d = kt - 4 * qr
                            if d >= 0:
                                eo = 512 * half + 128 * d
                                nc.vector.tensor_mul(stt[:, eo:eo + P],
                                                     stt[:, eo:eo + P], msk[:])
                        st[(h, qr, pi)] = stt

                pend_t = []

                def flush_t():
                    ph, pqr, zsbs = pend_t.pop(0)
                    # 4 transposes of one q-chunk gathered in a single PSUM
                    # tile (a bf16 bitcast view of an spp-tag slot, freeing a
                    # bank pair so QK pairs triple-buffer), one DVE eviction
                    ztf = sppool.tile([P, 1024], F32, tag='spp',
                                      name=f'ztp_{ph}_{pqr}')
                    ztq = ztf[:].bitcast(F16)
                    for j in range(4):
                        nc.tensor.transpose(ztq[:, j * P:(j + 1) * P],
                                            zsbs[j // 2][:, j % 2, :], ident[:])
                    nc.vector.tensor_copy(zT[ph][:, pqr * 512:(pqr + 1) * 512],
                                          ztq[:, 0:512])

                def emit_pv(h, qr):
                    zsbs = []
                    for qp in range(2):
                        zq = zppool.tile([P, 2, P + 1], F32, tag='zps',
                                         name=f'zps_{h}_{qr}_{qp}')
                        for qs2 in range(2):
                            qa = 4 * qr + 2 * qp + qs2
                            for kt in range(qa + 1):
                                stt = st[(h, qr, kt // 2)]
                                co = 512 * (kt & 1) + (qa - 4 * qr) * P
                                nc.tensor.matmul(zq[:, qs2, :],
                                                 stt[:, co:co + P],
                                                 vau_sb[kt][h & 1][:],
                                                 start=(kt == 0), stop=(kt == qa))
                        rcp2 = zpool.tile([P, 2], F32, tag='rcp',
                                          name=f'rcp_{h}_{qr}_{qp}')
                        nc.vector.reciprocal(rcp2[:], zq[:, :, P])
                        zsb2 = zpool.tile([P, 2, P], F16, tag='zsb',
                                          name=f'zsb_{h}_{qr}_{qp}')
                        nc.vector.tensor_scalar_mul(zsb2[:, 0, :], zq[:, 0, 0:P],
                                                    rcp2[:, 0:1])
                        nc.vector.tensor_scalar_mul(zsb2[:, 1, :], zq[:, 1, 0:P],
                                                    rcp2[:, 1:2])
                        zsbs.append(zsb2)
                    pend_t.append((h, qr, zsbs))
                    if len(pend_t) > 1:
                        flush_t()

                # flat group lists; QK emitted LOOKAHEAD groups ahead of PV
                groups = [(h, qr) for h in range(HG) for qr in range(NC512)]
                LOOKAHEAD = 4
                for j in range(LOOKAHEAD):
                    emit_qk(*groups[j])
                for i, (h, qr) in enumerate(groups):
                    # PV first: keeps ready PV chains ahead of exp-throttled
                    # QK matmuls in the in-order PE queue
                    emit_pv(h, qr)
                    if i + LOOKAHEAD < len(groups):
                        emit_qk(*groups[i + LOOKAHEAD])
                    if qr == 3 and h + 2 < HG:
                        fetch_vau(h + 2)

                while pend_t:
                    flush_t()

                for p in reversed(_ps2):
                    p.__exit__(None, None, None)

                # ================= stage 3: output projection =================
                with tc.tile_pool(name='s3p', bufs=4, space='PSUM') as oppool:
                    for ec in range(NC512):
                        es = slice(ec * 512, ec * 512 + 512)
                        wo = wopool.tile([P, HG, 512], F16, tag='wo', name=f'wo{ec}')
                        for h in range(HG):
                            nc.sync.dma_start(wo[:, h, :], woT[:, h, es])
                        for tt in range(NT):
                            pso = oppool.tile([P, 512], F32, tag='pso',
                                              name=f'pso_{tt}_{ec}')
                            for h in range(HG):
                                nc.tensor.matmul(pso[:], zT[h][:, tt * P:(tt + 1) * P],
                                                 wo[:, h, :],
                                                 start=(h == 0), stop=(h == HG - 1))
                            osb = ospool.tile([P, 512], F32, tag='osb',
                                              name=f'osb_{tt}_{ec}')
                            if tt % 2 == 0:
                                nc.scalar.copy(osb[:], pso[:])
                            else:
                                nc.vector.tensor_copy(osb[:], pso[:])
                            nc.sync.dma_start(out[tt][:, es], osb[:])

    nc.compile()
    return nc


def _host_inputs(x, Wqkv, Wo):
    """Build the 8 per-core input maps."""
    F16 = ml_dtypes.bfloat16
    # RoPE tables (match reference: float32 math)
    inv_freq = (1.0 / (BASE ** (np.arange(0, DH, 2, dtype=np.float32) / DH))).astype(np.float32)
    t = np.arange(S, dtype=np.float32)
    freqs = np.einsum('i,j->ij', t, inv_freq).astype(np.float32)   # [S, 64]
    emb = np.concatenate([freqs, freqs], axis=-1)                   # [S, 128]
    cos = np.cos(emb).astype(np.float32)
    sin = np.sin(emb).astype(np.float32)
    cosT = np.ascontiguousarray(cos.T)                              # [128, S]
    sinT = np.ascontiguousarray(sin.T)
    sinP = sinT.copy()
    sinP[0:64] = -sinP[0:64]

    # triangular causal mask [128, 128] f16: keep iff k_rel <= q_rel
    maskT = (np.arange(P)[:, None] <= np.arange(P)[None, :]).astype(F16)
    onesb = np.ones((P, 1), dtype=F16)

    in_maps = []
    for c in range(8):
        b, g = c // 2, c % 2
        heads = range(HG * g, HG * g + HG)
        x_b = x[b]                                       # [S, D]
        xT = np.ascontiguousarray(
            x_b.T.reshape(NDM, P, S).transpose(1, 0, 2)).astype(F16)  # [128, 16, S]
        # Q then K feature blocks, one per head in group
        blocks = [Wqkv[h * DH:(h + 1) * DH] for h in heads] + \
                 [Wqkv[D + h * DH:D + (h + 1) * DH] for h in heads]
        wqkT = np.stack([
            np.ascontiguousarray(
                blk.T.reshape(NDM, P, P).transpose(1, 0, 2))    # [128, 16, 128]
            for blk in blocks
        ]).astype(F16)                                           # [16, 128, 16, 128]
        Wv = np.concatenate([Wqkv[2 * D + h * DH:2 * D + (h + 1) * DH] for h in heads])
        wvT = np.ascontiguousarray(
            Wv.T.reshape(NDM, P, HG * P).transpose(1, 0, 2)).astype(F16)  # [128, 16, 1024]
        Wog = Wo[:, g * HG * DH:(g + 1) * HG * DH]               # [D, 1024]
        woT = np.ascontiguousarray(
            Wog.T.reshape(HG, P, D).transpose(1, 0, 2)).astype(F16)       # [128, 8, D]
        in_maps.append({
            'xT': xT, 'wqkT': wqkT, 'wvT': wvT, 'woT': woT,
            'cosT': cosT, 'sinP': sinP, 'maskT': maskT, 'onesb': onesb,
        })
    return in_maps


def kernel(x, Wqkv, Wo):
    from concourse.bass_utils import run_bass_kernel_spmd

    if 'nc' not in _CACHE:
        _CACHE['nc'] = _build_program()
    nc = _CACHE['nc']

    in_maps = _host_inputs(np.asarray(x, dtype=np.float32),
                           np.asarray(Wqkv, dtype=np.float32),
                           np.asarray(Wo, dtype=np.float32))
    res = run_bass_kernel_spmd(nc, in_maps, core_ids=list(range(8)))
    outs = [res.results[c]['out'].reshape(S, D) for c in range(8)]
    full = np.empty((B, S, D), dtype=np.float32)
    for b in range(B):
        full[b] = outs[2 * b] + outs[2 * b + 1]
    return full
